# revision 2
# baseline (speedup 1.0000x reference)
"""Bass/Trainium2 kernel for a 2-layer bidirectional GRU (PyTorch gate order).

Problem: B=32, T=512, I=512, H=512, L=2 bidirectional, fp32.

Sharding (v2): 8 cores = 4 batch quarters x 2 directions.  Each core
runs ONE direction for 8 batch rows, both layers.  Backward cores get
their inputs time-reversed by the host, so the on-device program is
direction-agnostic (always scans forward in local time) and fully SPMD.

Per core:
  1) gx0: input-gate activations for layer 0 (own direction).
  2) scan layer 0 -> h1T (bf16, local time order).
  3) AllReduce(add) over partner pairs {c, c+4}: ccs = h_fwd + h_bwd
     (each in its own local time order).
  4) gx1 = Wown . h_own(plain) + Wpar . rev(ccs) + (-Wpar) . rev(h_own)
     == Wown . h_own + Wpar . rev(h_partner); the reversal is done by
     giving the matmul a negative-stride moving-operand AP.
  5) scan layer 1 -> outT (bf16); host reverses bwd cores' outputs and
     concatenates halves.

Scan structure per step (single direction, BC=8):
  48 matmuls (12 gate tiles x 4 k) with N=8 moving columns; r,z gates
  first (contraction-outer so next step's first matmuls unblock on the
  first state half), then n gates tile-major so the n PSUM slices
  complete early; the n-path gate math is split into two k-halves so
  its serial tail overlaps the next step's matmuls.
"""

import numpy as np
import ml_dtypes

B, I, H = 32, 512, 512
T_FULL = 512
NCORES = 8
BC = 8                      # batch rows per core (4 quarters x 2 dirs)
NG = 12                     # 3H/128 gate-row tiles
KH = H // 128               # 4 k-tiles over H
KX = I // 128               # 4 k-tiles over I
TBLK = 32                   # scan steps per loop body

BF16 = ml_dtypes.bfloat16

_CACHE = {}


def _build_program(T, n_cores=NCORES):
    from contextlib import ExitStack
    import concourse.mybir as mybir
    import concourse.tile as tile
    from concourse import bacc
    from concourse.bass import ds

    bf = mybir.dt.bfloat16
    f32 = mybir.dt.float32
    ADD = mybir.AluOpType.add
    MUL = mybir.AluOpType.mult
    SIG = mybir.ActivationFunctionType.Sigmoid
    TANH = mybir.ActivationFunctionType.Tanh

    NBODY = T // TBLK
    HB = TBLK // 2
    assert T % TBLK == 0

    nc = bacc.Bacc("TRN2", target_bir_lowering=False, debug=False,
                   enable_asserts=False, num_devices=n_cores)

    # ---- DRAM tensors (per-core shard) ----
    xTb = nc.dram_tensor("xTb", [128, KX, BC, T], bf, kind="ExternalInput").ap()
    wih0 = nc.dram_tensor("wih0", [128, KX, NG, 128], bf, kind="ExternalInput").ap()
    wih1 = nc.dram_tensor("wih1", [128, 3, KH, NG, 128], bf, kind="ExternalInput").ap()
    whh = nc.dram_tensor("whh", [128, 2, KH, NG, 128], bf, kind="ExternalInput").ap()
    bgx = nc.dram_tensor("bgx", [128, 2, NG], f32, kind="ExternalInput").ap()
    bhn = nc.dram_tensor("bhn", [128, 2, KH, BC], f32, kind="ExternalInput").ap()
    gxd0 = nc.dram_tensor("gxd0", [NG, BC, 128, T + TBLK], f32, kind="Internal").ap()
    gxd1 = nc.dram_tensor("gxd1", [NG, BC, 128, T + TBLK], f32, kind="Internal").ap()
    h1T = nc.dram_tensor("h1T", [128, KH, BC, T], bf, kind="Internal").ap()
    ccs = nc.dram_tensor("ccs", [128, KH, BC, T], bf, kind="Internal").ap()
    outT = nc.dram_tensor("outT", [128, KH, BC, T], bf, kind="ExternalOutput").ap()

    with tile.TileContext(nc) as tc:
        with tc.tile_pool(name="persist", bufs=1) as persist:
            whh_sb = persist.tile([128, 2, KH, NG, 128], bf)
            nc.sync.dma_start(out=whh_sb, in_=whh)
            bhn_sb = persist.tile([128, 2, KH, BC], f32, tag="bhn")
            nc.sync.dma_start(out=bhn_sb, in_=bhn)
            bgx_sb = persist.tile([128, 2, NG], f32, tag="bgx")
            nc.sync.dma_start(out=bgx_sb, in_=bgx)

            # ------------- gx phases -------------
            def gx0_phase():
                with tc.tile_pool(name="gxw", bufs=1) as gxw, \
                     tc.tile_pool(name="gxmov", bufs=2 * KX) as gxmov, \
                     tc.tile_pool(name="gxps", bufs=4, space="PSUM") as gxps, \
                     tc.tile_pool(name="gxcp", bufs=4) as gxcp:
                    wih_sb = gxw.tile([128, KX, NG, 128], bf)
                    nc.sync.dma_start(out=wih_sb, in_=wih0)
                    for b in range(BC):
                        movs = []
                        for k in range(KX):
                            mv = gxmov.tile([128, T], bf, tag="mov")
                            nc.sync.dma_start(out=mv, in_=xTb[:, k, b, :])
                            movs.append(mv)
                        for gt in range(NG):
                            ps = gxps.tile([128, T], f32, tag="ps")
                            for k in range(KX):
                                nc.tensor.matmul(ps, wih_sb[:, k, gt, :], movs[k],
                                                 start=(k == 0), stop=(k == KX - 1))
                            cp = gxcp.tile([128, T], f32, tag="cp")
                            nc.vector.tensor_scalar(
                                out=cp, in0=ps, scalar1=bgx_sb[:, 0, gt:gt + 1],
                                scalar2=None, op0=ADD)
                            nc.sync.dma_start(out=gxd0[gt, b, :, 0:T], in_=cp)

            def gx1_phase():
                with tc.tile_pool(name="g1w", bufs=1) as g1w, \
                     tc.tile_pool(name="g1mov", bufs=4 * KH) as g1mov, \
                     tc.tile_pool(name="g1ps", bufs=4, space="PSUM") as g1ps, \
                     tc.tile_pool(name="g1cp", bufs=4) as g1cp:
                    w1_sb = g1w.tile([128, 3, KH, NG, 128], bf)
                    nc.sync.dma_start(out=w1_sb, in_=wih1)
                    for b in range(BC):
                        movA, movS = [], []
                        for k in range(KH):
                            mv = g1mov.tile([128, T], bf, tag="movA")
                            nc.sync.dma_start(out=mv, in_=h1T[:, k, b, :])
                            movA.append(mv)
                        for k in range(KH):
                            mv = g1mov.tile([128, T], bf, tag="movS")
                            nc.sync.dma_start(out=mv, in_=ccs[:, k, b, :])
                            movS.append(mv)
                        for gt in range(NG):
                            ps = g1ps.tile([128, T], f32, tag="ps")
                            n_mm = 3 * KH
                            i = 0
                            for k in range(KH):
                                nc.tensor.matmul(ps, w1_sb[:, 0, k, gt, :], movA[k],
                                                 start=(i == 0), stop=(i == n_mm - 1))
                                i += 1
                            for k in range(KH):
                                nc.tensor.matmul(ps, w1_sb[:, 1, k, gt, :],
                                                 movS[k][:, ::-1],
                                                 start=(i == 0), stop=(i == n_mm - 1))
                                i += 1
                            for k in range(KH):
                                nc.tensor.matmul(ps, w1_sb[:, 2, k, gt, :],
                                                 movA[k][:, ::-1],
                                                 start=(i == 0), stop=(i == n_mm - 1))
                                i += 1
                            cp = g1cp.tile([128, T], f32, tag="cp")
                            nc.vector.tensor_scalar(
                                out=cp, in0=ps, scalar1=bgx_sb[:, 1, gt:gt + 1],
                                scalar2=None, op0=ADD)
                            nc.sync.dma_start(out=gxd1[gt, b, :, 0:T], in_=cp)

            # ------------- scan phase -------------
            def scan_phase(l, gxd, out_dram):
                with tc.tile_pool(name="sblk", bufs=1) as sblk_pool, \
                     tc.tile_pool(name="gxblk", bufs=1) as gxblk_pool, \
                     tc.tile_pool(name="psrz0", bufs=2, space="PSUM") as psrz0_pool, \
                     tc.tile_pool(name="psrz1", bufs=2, space="PSUM") as psrz1_pool, \
                     tc.tile_pool(name="psn", bufs=4, space="PSUM") as psn_pool, \
                     tc.tile_pool(name="stemp", bufs=4) as tp:
                    s32 = sblk_pool.tile([128, KH, BC, TBLK], f32, tag="s32")
                    s16 = sblk_pool.tile([128, KH, BC, TBLK], bf, tag="s16")
                    gxA = gxblk_pool.tile([128, NG, BC, HB], f32, tag="gxA")
                    gxB = gxblk_pool.tile([128, NG, BC, HB], f32, tag="gxB")
                    nc.vector.memset(s32, 0.0)
                    nc.vector.memset(s16, 0.0)
                    # prologue: first body's A half (steps 0..HB-1)
                    nc.sync.dma_start(
                        out=gxA,
                        in_=gxd[:, :, :, 0:HB].rearrange("g b p t -> p g b t"))

                    with tc.For_i(0, NBODY, 1,
                                  hint_engines=(mybir.EngineType.PE,
                                                mybir.EngineType.DVE)) as iv:
                        nc.sync.dma_start(
                            out=gxB,
                            in_=gxd[:, :, :, ds(iv * TBLK + HB, HB)]
                            .rearrange("g b p t -> p g b t"))
                        for j in range(TBLK):
                            jj = j
                            pj = (j - 1) % TBLK
                            if j == HB:
                                # prefetch next body's A half
                                nc.sync.dma_start(
                                    out=gxA,
                                    in_=gxd[:, :, :, ds((iv + 1) * TBLK, HB)]
                                    .rearrange("g b p t -> p g b t"))
                            gxt = gxA if j < HB else gxB
                            qq = j % HB
                            # full-bank (2KB/partition) PSUM tiles so each
                            # lives in its own bank: PE-write vs DVE-read of
                            # the same bank is a fatal HW error
                            ps_rz0f = psrz0_pool.tile([128, 8, 64], f32,
                                                      tag="psrz0")
                            ps_rz1f = psrz1_pool.tile([128, 8, 64], f32,
                                                      tag="psrz1")
                            ps_nf = psn_pool.tile([128, KH, 128], f32,
                                                  tag="psn")
                            ps_rz0 = ps_rz0f[:, :, 0:BC]
                            ps_rz1 = ps_rz1f[:, :, 0:BC]
                            ps_n = ps_nf[:, :, 0:BC]
                            # r,z matmuls, contraction split across two PSUM
                            # banks: the k=0,1 half only needs state half 0 of
                            # the previous step, so it can issue while the
                            # previous step's half-1 tail is still running.
                            # (Groups sharing a bank must stay contiguous:
                            # start=True clears has_written for the whole
                            # bank.)
                            for half, ps_h in ((0, ps_rz0), (1, ps_rz1)):
                                for gt in range(8):
                                    for k in (2 * half, 2 * half + 1):
                                        nc.tensor.matmul(
                                            ps_h[:, gt, :],
                                            whh_sb[:, l, k, gt, :],
                                            s16[:, k, :, pj],
                                            start=(k == 2 * half),
                                            stop=(k == 2 * half + 1))
                            # n matmuls: tile-major so each PSUM slice
                            # finishes as early as possible
                            for gt in range(8, NG):
                                for k in range(KH):
                                    nc.tensor.matmul(
                                        ps_n[:, gt - 8, :],
                                        whh_sb[:, l, k, gt, :],
                                        s16[:, k, :, pj],
                                        start=(k == 0), stop=(k == KH - 1))
                            # r,z pre-activations and gates (a TensorTensor
                            # may read only one PSUM input, so associate as
                            # (ps0 + gx) + ps1)
                            rzadd = tp.tile([128, 8, BC], f32, tag="rzadd")
                            nc.vector.tensor_tensor(rzadd, ps_rz0,
                                                    gxt[:, 0:8, :, qq], ADD)
                            rzin = tp.tile([128, 8, BC], f32, tag="rzin")
                            nc.vector.tensor_tensor(rzin, rzadd, ps_rz1, ADD)
                            sig = tp.tile([128, 8, BC], f32, tag="sig")
                            nc.scalar.activation(sig, rzin, SIG)
                            omz = tp.tile([128, KH, BC], f32, tag="omz")
                            nc.scalar.activation(omz, rzin[:, 4:8, :], SIG,
                                                 scale=-1.0)
                            zh = tp.tile([128, KH, BC], f32, tag="zh")
                            nc.gpsimd.tensor_tensor(zh, sig[:, 4:8, :],
                                                    s32[:, :, :, pj], MUL)
                            hn2 = tp.tile([128, KH, BC], f32, tag="hn2")
                            nc.vector.tensor_tensor(hn2, ps_n, bhn_sb[:, l], ADD)
                            nom = tp.tile([128, KH, BC], f32, tag="nom")
                            # n-path in two k-halves: the half-1 tail overlaps
                            # the next step's first matmuls (which need only
                            # state half 0)
                            for s in range(2):
                                sl = slice(2 * s, 2 * s + 2)
                                nm = tp.tile([128, 2, BC], f32, tag=f"nm{s}")
                                nc.vector.tensor_tensor(nm, sig[:, sl, :],
                                                        hn2[:, sl, :], MUL)
                                nin = tp.tile([128, 2, BC], f32, tag=f"nin{s}")
                                nc.vector.tensor_tensor(
                                    nin, nm, gxt[:, 8 + 2 * s:10 + 2 * s, :, qq],
                                    ADD)
                                n = tp.tile([128, 2, BC], f32, tag=f"n{s}")
                                nc.scalar.activation(n, nin, TANH)
                                nc.gpsimd.tensor_tensor(nom[:, sl, :], n,
                                                        omz[:, sl, :], MUL)
                                nc.vector.tensor_tensor(s16[:, sl, :, jj],
                                                        nom[:, sl, :],
                                                        zh[:, sl, :], ADD)
                            nc.gpsimd.tensor_tensor(s32[:, :, :, jj], nom, zh,
                                                    ADD)
                        nc.sync.dma_start(
                            out=out_dram[:, :, :, ds(iv * TBLK, TBLK)], in_=s16)

            gx0_phase()
            scan_phase(0, gxd0, h1T)
            nc.gpsimd.collective_compute(
                "AllReduce", mybir.AluOpType.add,
                replica_groups=[[0, 4], [1, 5], [2, 6], [3, 7]],
                ins=[h1T], outs=[ccs])
            gx1_phase()
            scan_phase(1, gxd1, outT)

    nc.compile()
    return nc


def _get_program(T):
    if T not in _CACHE:
        _CACHE[T] = _build_program(T)
    return _CACHE[T]


def _wtile_prep(w, K):
    """w: [3H, K*128] -> [128p, Kk, NGgt, 128c] (c = gate col, p = in-row)."""
    a = np.ascontiguousarray(w.T).reshape(K, 128, NG, 128)
    return np.ascontiguousarray(np.transpose(a, (1, 0, 2, 3))).astype(BF16)


def _prep_dir(d, w_ih_l0, w_hh_l0, b_ih_l0, b_hh_l0,
              w_ih_l1, w_hh_l1, b_ih_l1, b_hh_l1):
    W1 = w_ih_l1[d]
    own = W1[:, :H] if d == 0 else W1[:, H:]
    par = W1[:, H:] if d == 0 else W1[:, :H]
    wih1 = np.stack([_wtile_prep(own, KH), _wtile_prep(par, KH),
                     _wtile_prep(-par, KH)], axis=1)
    whh = np.stack([_wtile_prep(w_hh_l0[d], KH),
                    _wtile_prep(w_hh_l1[d], KH)], axis=1)

    g = np.arange(3 * H)
    rz_mask = (g < 2 * H).astype(np.float32)

    def bgx_prep(b_ih, b_hh):
        v = b_ih[d] + b_hh[d] * rz_mask
        return np.ascontiguousarray(v.reshape(NG, 128).T)

    bgx = np.stack([bgx_prep(b_ih_l0, b_hh_l0),
                    bgx_prep(b_ih_l1, b_hh_l1)], axis=1).astype(np.float32)

    def bhn_prep(b_hh):
        v = b_hh[d][2 * H:].reshape(KH, 128).T    # [128, KH]
        return np.broadcast_to(v[:, :, None], (128, KH, BC))

    bhnv = np.ascontiguousarray(
        np.stack([bhn_prep(b_hh_l0), bhn_prep(b_hh_l1)],
                 axis=1)).astype(np.float32)

    return {"wih0": _wtile_prep(w_ih_l0[d], KX), "wih1": wih1, "whh": whh,
            "bgx": np.ascontiguousarray(bgx), "bhn": bhnv}


def kernel(x, w_ih_l0, w_hh_l0, b_ih_l0, b_hh_l0,
           w_ih_l1, w_hh_l1, b_ih_l1, b_hh_l1, _trace=False):
    from concourse.bass_utils import run_bass_kernel_spmd

    x = np.asarray(x, dtype=np.float32)
    T = x.shape[1]
    args = [np.asarray(a, np.float32) for a in
            (w_ih_l0, w_hh_l0, b_ih_l0, b_hh_l0,
             w_ih_l1, w_hh_l1, b_ih_l1, b_hh_l1)]
    shared = {d: _prep_dir(d, *args) for d in range(2)}

    in_maps = []
    for c in range(NCORES):
        d, q = c // 4, c % 4
        xs = x[q * BC:(q + 1) * BC]                      # [BC, T, I]
        if d == 1:
            xs = xs[:, ::-1]
        a = np.transpose(xs, (2, 0, 1)).reshape(KX, 128, BC, T)
        xtb = np.ascontiguousarray(np.transpose(a, (1, 0, 2, 3))).astype(BF16)
        m = dict(shared[d])
        m["xTb"] = xtb
        in_maps.append(m)

    nc = _get_program(T)
    res = run_bass_kernel_spmd(nc, in_maps, core_ids=list(range(NCORES)),
                               trace=_trace)

    out = np.empty((B, T, 2 * H), dtype=np.float32)
    for c in range(NCORES):
        d, q = c // 4, c % 4
        o = res.results[c]["outT"]                       # [128, KH, BC, T]
        o = np.transpose(o, (2, 3, 1, 0)).reshape(BC, T, H).astype(np.float32)
        if d == 1:
            o = o[:, ::-1]
        out[q * BC:(q + 1) * BC, :, d * H:(d + 1) * H] = o
    if _trace:
        kernel._last_results = res
    return out
